# revision 5
# baseline (speedup 1.0000x reference)
"""GNN message-passing on 8 Trainium2 cores — gather + segment-sum matmul.

Per core e (edge-type sharding):
  phase A: proj_e = x @ W_e^T   (PE fp16 in, f32 PSUM, fp16 out) -> DRAM
  phase B: edges grouped (src_bucket, tgt_window); dma_gather proj rows
           (1024/call, the HW limit); aggregate per 128-target window with
           one-hot selection matmuls on the PE:
             S_strip[p, :] = (iota == goff[p, col])      (DVE is_equal, fp16)
             psum[w] += S_strip[:, k*128:...]^T @ gathered_col
           flush each (bucket-run, window) PSUM tile -> fp16 -> DRAM.
  host:    sum bucket partials per window, add bias*counts_e, divide by
           global in-degree. (Cross-core reduce on host, as the baseline.)

The pass/flush schedule is SPMD-static: per-(bucket, window) slot segments
sized cap = max over cores; padding slots gather row 0 with off=PADOFF so
the one-hot never matches (contributes exactly 0).
"""

import numpy as np

import concourse.bacc as bacc
import concourse.mybir as mybir
import concourse.tile as tile
from concourse.bass_utils import run_bass_kernel_spmd

N = 100000    # nodes
D = 128       # hidden
E = 8         # edge types == cores
M = 200000    # edges per type

TW = 128      # targets per window
BL = 2        # windows per harmonization block (256 targets)
CALL = 1024   # max idxs per dma_gather (HW limit)
KMAX = 6      # max windows a 128-slot column may straddle
PADOFF = 2000.0

NPAD = -(-N // 512) * 512
BS = NPAD // 4
NB = 4
NWIN = NPAD // TW
STRIP = KMAX * TW

F32 = mybir.dt.float32
F16 = mybir.dt.float16
I16 = mybir.dt.int16

TRACE = False
LAST = None


def _derive(n, m):
    global N, M, NPAD, BS, NB, NWIN, STRIP
    N, M = n, m
    NPAD = -(-N // 512) * 512
    BS = NPAD // 4
    NWIN = NPAD // TW
    STRIP = KMAX * TW


def build_schedule(edge_lists):
    """Static layout shared by all cores + per-core index/offset data."""
    src = np.asarray(edge_lists[:, :, 0], dtype=np.int64)
    tgt = np.asarray(edge_lists[:, :, 1], dtype=np.int64)
    sb = src // BS
    wq = tgt // TW

    NBLK = -(-NWIN // BL)                           # harmonization blocks
    blk = wq // BL
    counts = np.zeros((E, NB, NBLK), dtype=np.int64)
    for e in range(E):
        np.add.at(counts[e], (sb[e], blk[e]), 1)
    caps = counts.max(axis=0)                       # [NB, NBLK]

    # slot layout: runs by sb, segments by block; run length padded to x128
    seg_start = np.zeros((NB, NBLK), dtype=np.int64)
    run_start = np.zeros(NB + 1, dtype=np.int64)
    pos = 0
    for b in range(NB):
        run_start[b] = pos
        for bi in range(NBLK):
            seg_start[b, bi] = pos
            pos += caps[b, bi]
        pos = -(-pos // 128) * 128
    run_start[NB] = pos
    tot = pos
    ncol = tot // 128

    seg_flat_start = seg_start.reshape(-1)
    seg_flat_end = seg_flat_start + caps.reshape(-1)

    # per-column static meta: (bucket, base window, K windows)
    col_meta = []
    for c in range(ncol):
        lo, hi = c * 128, c * 128 + 128
        b = int(np.searchsorted(run_start[1:], lo, side="right"))
        s = seg_flat_start[b * NBLK:(b + 1) * NBLK]
        t = seg_flat_end[b * NBLK:(b + 1) * NBLK]
        blks = np.flatnonzero((s < hi) & (t > lo))
        if len(blks) == 0:
            col_meta.append((b, 0, 0))
            continue
        wb = int(blks[0]) * BL
        wl = min(int(blks[-1]) * BL + BL - 1, NWIN - 1)
        K = wl - wb + 1
        assert K <= KMAX, f"col {c}: K={K} > KMAX"
        col_meta.append((b, wb, K))

    # first/last column touching each (b, w); bidx = dense flush index
    first_col, last_col = {}, {}
    for c, (b, wb, K) in enumerate(col_meta):
        for k in range(K):
            key = (b, wb + k)
            first_col.setdefault(key, c)
            last_col[key] = c
    # bidx assigned in FLUSH order (program order of last_col events) so
    # flushed tiles can be staged and DMA'd out in contiguous batches
    flush_events = []                               # (col, k) -> key
    for c, (b, wb, K) in enumerate(col_meta):
        for k in range(K):
            key = (b, wb + k)
            if last_col[key] == c:
                flush_events.append(key)
    bidx_map = {key: i for i, key in enumerate(flush_events)}
    nflush = len(bidx_map)

    # per-core slot data
    wb_per_col = np.array([m_[1] for m_ in col_meta], dtype=np.int64)
    gsrc = np.zeros((E, tot), dtype=np.int16)
    goff = np.full((E, 128, ncol), PADOFF, dtype=np.float32)
    for e in range(E):
        order = np.lexsort((tgt[e], blk[e], sb[e]))
        se, te = src[e][order], tgt[e][order]
        key = sb[e][order] * NBLK + blk[e][order]
        grp_first = np.searchsorted(key, key, side="left")
        rank = np.arange(M) - grp_first
        slot = seg_flat_start[key] + rank
        gsrc[e, slot] = (se % BS).astype(np.int16)
        col = slot // 128
        p = slot % 128
        goff[e, p, col] = (te - wb_per_col[col] * TW).astype(np.float32)
    assert goff[goff != PADOFF].max(initial=0) < STRIP

    gsrc_w = np.ascontiguousarray(
        np.tile(gsrc.reshape(E, -1, 16).transpose(0, 2, 1), (1, 8, 1))
    )
    return dict(
        caps=caps, gsrc_w=gsrc_w, goff=goff.astype(np.float16),
        col_meta=col_meta, first_col=first_col, last_col=last_col,
        bidx_map=bidx_map, tot=tot, nflush=nflush, run_start=run_start,
    )


def build_bass(sched):
    col_meta = sched["col_meta"]
    first_col, last_col = sched["first_col"], sched["last_col"]
    bidx_map = sched["bidx_map"]
    tot, nflush = sched["tot"], sched["nflush"]
    run_start = sched["run_start"]
    ncol = tot // 128

    nc = bacc.Bacc("TRN2", target_bir_lowering=False)

    xt_d = nc.dram_tensor("xt", [D, NPAD], F16, kind="ExternalInput")
    wt_d = nc.dram_tensor("wt", [D, D], F16, kind="ExternalInput")
    gs_d = nc.dram_tensor("gsrc", [128, tot // 16], I16, kind="ExternalInput")
    go_d = nc.dram_tensor("goff", [128, ncol], F16, kind="ExternalInput")
    iota_d = nc.dram_tensor("iota", [128, STRIP], F16, kind="ExternalInput")
    proj_d = nc.dram_tensor("proj", [NPAD, D], F16)
    msg_d = nc.dram_tensor("msg", [nflush * 128, D], F16, kind="ExternalOutput")

    with tile.TileContext(nc) as tc:
        with (
            tc.tile_pool(name="const", bufs=1) as constp,
            tc.tile_pool(name="xtp", bufs=3) as xtp,
            tc.tile_pool(name="pout", bufs=3) as pop,
            tc.tile_pool(name="psA", bufs=2, space="PSUM") as psA,
            tc.tile_pool(name="gat", bufs=6) as gp,
            tc.tile_pool(name="strip", bufs=6) as sp,
            tc.tile_pool(name="psB", bufs=6, space="PSUM") as psB,
            tc.tile_pool(name="fl", bufs=6) as fp,
        ):
            wt_s = constp.tile([D, D], F16)
            nc.sync.dma_start(wt_s[:], wt_d[:])
            iota_s = constp.tile([128, STRIP], F16)
            nc.sync.dma_start(iota_s[:], iota_d[:])
            goff_s = constp.tile([128, ncol], F16)
            nc.sync.dma_start(goff_s[:], go_d[:])
            gs_s = constp.tile([128, tot // 16], I16)
            nc.sync.dma_start(gs_s[:], gs_d[:])

            # ---- Phase A: proj = x @ W^T (fp16) ----
            # 4 node-chunks per PSUM bank; one wide PSUM->SBUF copy per bank,
            # alternating Act/DVE so bucket 0 is ready for gathers ASAP.
            XTCH = 4096

            def phase_a_chunk(n0, nch):
                xt_t = xtp.tile([128, XTCH], F16, tag="xt")
                nc.sync.dma_start(xt_t[:, :nch], xt_d[:, n0:n0 + nch])
                ob = pop.tile([128, XTCH // 128, D], F16, tag="pout")
                for c4 in range(0, nch // 128, 4):
                    nb4 = min(4, nch // 128 - c4)
                    pA = psA.tile([128, 4, D], F32, tag="psA")
                    for ci in range(c4, c4 + nb4):
                        nc.tensor.matmul(
                            pA[:, ci - c4, :],
                            xt_t[:, ci * 128:(ci + 1) * 128], wt_s[:],
                            start=True, stop=True,
                        )
                    if (c4 // 4) % 2 == 0:
                        nc.scalar.activation(
                            ob[:, c4:c4 + nb4, :], pA[:, :nb4, :],
                            mybir.ActivationFunctionType.Copy,
                        )
                    else:
                        nc.vector.tensor_copy(
                            ob[:, c4:c4 + nb4, :], pA[:, :nb4, :]
                        )
                nc.sync.dma_start(
                    proj_d[n0:n0 + nch, :].rearrange("(c p) d -> p c d", p=128),
                    ob[:, : nch // 128, :],
                )

            # bucket 0's proj first (needed by the first gathers); the rest
            # of phase A is sprinkled one chunk per gather call so the PE
            # FIFO never backs up segment matmuls behind a phase-A blob
            # (which stalls gather-tile recycling -> Pool bubbles).
            for n0 in range(0, BS, XTCH):
                phase_a_chunk(n0, min(XTCH, BS - n0))
            rest_chunks = [
                (n0, min(XTCH, NPAD - n0)) for n0 in range(BS, NPAD, XTCH)
            ]
            rest_i = 0

            # ---- Phase B: gather + segment matmuls ----
            FB = 8
            psum_tiles = {}
            stage = None
            for b in range(NB):
                lo, hi = int(run_start[b]), int(run_start[b + 1])
                for off in range(lo, hi, CALL):
                    if rest_i < len(rest_chunks):
                        phase_a_chunk(*rest_chunks[rest_i])
                        rest_i += 1
                    sz = min(CALL, hi - off)
                    g = gp.tile([128, CALL // 128, D], F16, tag="gat")
                    nc.gpsimd.dma_gather(
                        g[:, : sz // 128, :], proj_d[b * BS:(b + 1) * BS, :],
                        gs_s[:, off // 16:(off + sz) // 16], sz, sz, D,
                        queue_num=0, single_packet=False,
                    )
                    for cc in range(off // 128, (off + sz) // 128):
                        bb, wb, K = col_meta[cc]
                        if K == 0:
                            continue
                        gci = cc - off // 128
                        strip = sp.tile([128, STRIP], F16, tag="strip")
                        nc.vector.tensor_tensor(
                            strip[:, : K * TW],
                            iota_s[:, : K * TW],
                            goff_s[:, cc:cc + 1].broadcast_to([128, K * TW]),
                            mybir.AluOpType.is_equal,
                        )
                        for k in range(K):
                            key = (bb, wb + k)
                            if first_col[key] == cc:
                                pt = psB.tile([128, D], F32, tag="psB")
                                psum_tiles[key] = pt
                            else:
                                pt = psum_tiles[key]
                            is_last = last_col[key] == cc
                            nc.tensor.matmul(
                                pt[:], strip[:, k * TW:(k + 1) * TW],
                                g[:, gci, :],
                                start=(first_col[key] == cc), stop=is_last,
                            )
                            if is_last:
                                pt = psum_tiles.pop(key)
                                bidx = bidx_map[key]
                                sl = bidx % FB
                                if sl == 0:
                                    stage = fp.tile([128, FB, D], F16, tag="fl")
                                nc.scalar.activation(
                                    stage[:, sl, :], pt[:],
                                    mybir.ActivationFunctionType.Copy,
                                )
                                if sl == FB - 1 or bidx == nflush - 1:
                                    b0 = bidx - sl
                                    nc.sync.dma_start(
                                        msg_d[b0 * 128:(b0 + sl + 1) * 128, :]
                                        .rearrange("(c p) d -> p c d", p=128),
                                        stage[:, : sl + 1, :],
                                    )
    nc.compile()
    return nc


def kernel(edge_lists, node_states, W=None, b=None, **kw):
    global LAST
    W_in = W if W is not None else kw["W"]
    b_in = b if b is not None else kw["b"]
    edge_lists = np.asarray(edge_lists)
    x = np.asarray(node_states, dtype=np.float32)
    Wm = np.asarray(W_in, dtype=np.float32)
    bv = np.asarray(b_in, dtype=np.float32)

    sched = build_schedule(edge_lists)
    nc = build_bass(sched)

    xt = np.zeros((D, NPAD), dtype=np.float32)
    xt[:, :N] = x.T
    xt = xt.astype(np.float16)
    iota = np.ascontiguousarray(
        np.broadcast_to(np.arange(STRIP, dtype=np.float16), (128, STRIP))
    )

    in_maps = []
    for e in range(E):
        wt = np.ascontiguousarray(Wm[e * D:(e + 1) * D, :].T).astype(np.float16)
        in_maps.append(
            {
                "xt": xt,
                "wt": wt,
                "gsrc": sched["gsrc_w"][e],
                "goff": np.ascontiguousarray(sched["goff"][e]),
                "iota": iota,
            }
        )

    res = run_bass_kernel_spmd(
        nc, in_maps, core_ids=list(range(E)), trace=TRACE
    )
    LAST = res

    tgt_all = np.asarray(edge_lists[:, :, 1], dtype=np.int64)
    total = np.zeros((NPAD, D), dtype=np.float32)
    bidx_map = sched["bidx_map"]
    for e in range(E):
        msg = np.asarray(res.results[e]["msg"]).astype(np.float32)
        for (b_, wi), bidx in bidx_map.items():
            total[wi * TW:(wi + 1) * TW] += msg[bidx * 128:(bidx + 1) * 128]
        ce = np.bincount(tgt_all[e], minlength=NPAD).astype(np.float32)
        total += ce[:, None] * bv[e * D:(e + 1) * D][None, :]

    counts = np.bincount(tgt_all.reshape(-1), minlength=NPAD).astype(np.float32)
    divisor = np.where(counts == 0.0, 1.0, counts)
    return (total / divisor[:, None])[:N].astype(np.float32)
